# revision 6
# baseline (speedup 1.0000x reference)
"""Trainium2 Bass kernel for nn_Head_37623913513539.

Computation (per batch b):
    q = x @ Wd_w.T + Wd_b                    [T, L]
    h = causal_mask(q @ Wdkv.T / 8)          [T, T]
    y = softmax(h, axis=-1)
    out = y @ Wdkv                           [T, L]

Strategy: pure data parallelism across 8 NeuronCores, no collectives.
Each core owns half of one batch's queries: 8 chunks of 256 query rows,
chosen so both halves have identical causal work (36 key-tiles actual,
40 programmed). All cores run ONE uniform SPMD program; per-core
differences (which chunks, causal thresholds) are carried in the data:

  - scores are computed transposed (keys on partitions, queries on the
    free axis) so no on-device transposes are needed anywhere;
  - softmax max-subtraction is skipped (|h/8| <= ~14, exp is safe in f32);
  - the softmax denominator comes free as an extra all-ones column
    appended to Wdkv in the PV matmul;
  - causality is applied by comparing a resident D[s,t] = t-s tile
    against a per-(slot, key-tile) threshold theta streamed per core;
    fully-masked "padding" key-tiles come out all-zero automatically.

All matmuls are bf16 with f32 PSUM accumulation (validated ~0.8%
scale-relative absmax error vs the f32 reference).
"""

import os
import sys

import numpy as np
import ml_dtypes

for _p in ("/opt/trn_rl_repo",):
    if _p not in sys.path and os.path.isdir(_p):
        sys.path.insert(0, _p)

from contextlib import ExitStack

import concourse.bass as bass
import concourse.mybir as mybir
import concourse.tile as tile
from concourse import bacc
from concourse.bass_utils import run_bass_kernel_spmd

BF16 = ml_dtypes.bfloat16

B, T, C, L = 4, 2048, 1024, 288
P = 128
CHUNK = 256                      # query rows per chunk
NSLOT = 4                        # chunks per core
TLOC = NSLOT * CHUNK             # 1024 query rows per core
SLOTS = [4, 8, 12, 16]           # programmed key-tiles per slot
NTHETA = sum(SLOTS)              # 40
LA = L + 1                       # 289: extra ones-column for the denominator
CHUNKS_H = [[1, 2, 5, 6], [0, 3, 4, 7]]  # per-half chunk assignment
N_CORES = 8

_cached_nc = None


def _build_program():
    """Emit the uniform single-core program (same NEFF for all 8 cores)."""
    nc = bacc.Bacc(None)
    f32 = mybir.dt.float32
    bf = mybir.dt.bfloat16

    xT = nc.declare_dram_parameter("xT", [C, TLOC], bf, isOutput=False)
    wdT = nc.declare_dram_parameter("wdT", [C, L], bf, isOutput=False)
    bias = nc.declare_dram_parameter("bias", [P, 3], f32, isOutput=False)
    kvT = nc.declare_dram_parameter("kvT", [3 * P, T], bf, isOutput=False)
    kva = nc.declare_dram_parameter("kva", [T, LA], bf, isOutput=False)
    dmat = nc.declare_dram_parameter("dmat", [P, CHUNK], bf, isOutput=False)
    theta = nc.declare_dram_parameter("theta", [P, NTHETA], f32, isOutput=False)
    out = nc.declare_dram_parameter("out", [TLOC, L], f32, isOutput=True)

    Exp = mybir.ActivationFunctionType.Exp
    Ident = mybir.ActivationFunctionType.Identity

    with tile.TileContext(nc) as tc, ExitStack() as ctx:
        consts = ctx.enter_context(tc.tile_pool(name="consts", bufs=1))
        sb_y = ctx.enter_context(tc.tile_pool(name="ytiles", bufs=4))
        sb_m = ctx.enter_context(tc.tile_pool(name="mtiles", bufs=4))
        sb_o = ctx.enter_context(tc.tile_pool(name="otiles", bufs=3))
        sb_r = ctx.enter_context(tc.tile_pool(name="rtiles", bufs=3))
        ps_q = ctx.enter_context(tc.tile_pool(name="psq", bufs=2, space="PSUM"))
        ps_h = ctx.enter_context(tc.tile_pool(name="psh", bufs=3, space="PSUM"))
        ps_o = ctx.enter_context(tc.tile_pool(name="pso", bufs=1, space="PSUM"))

        # ---- resident tiles + loads -------------------------------------
        wdT_sb = consts.tile([P, C // P, L], bf)       # [c-part, ct, l]
        bias_sb = consts.tile([P, 3], f32)
        dmat_sb = consts.tile([P, CHUNK], bf)
        theta_sb = consts.tile([P, NTHETA], f32)
        # x.T halves split by query column for qproj/DMA overlap
        xh = [consts.tile([P, C // P, TLOC // 2], bf, tag=f"xh{i}", name=f"xh{i}")
              for i in range(2)]
        qT_sb = consts.tile([P, 3, TLOC], bf)          # [l-part, lt, t]
        kvT_sb = consts.tile([P, 3, T], bf)            # [l-part, lt, s]
        kva_sb = consts.tile([P, T // P, LA], bf)      # [s-part, st, l]

        nc.sync.dma_start(wdT_sb[:], wdT[:].rearrange("(ct p) l -> p ct l", p=P))
        nc.sync.dma_start(bias_sb[:], bias[:])
        nc.sync.dma_start(dmat_sb[:], dmat[:])
        nc.sync.dma_start(theta_sb[:], theta[:])
        xT_r = xT[:].rearrange("(ct p) t -> p ct t", p=P)
        for i in range(2):
            nc.sync.dma_start(xh[i][:], xT_r[:, :, i * 512:(i + 1) * 512])
        kvT_r = kvT[:].rearrange("(lt p) s -> p lt s", p=P)
        nc.sync.dma_start(kvT_sb[:, 0:2, :], kvT_r[:, 0:2, :])
        nc.sync.dma_start(kvT_sb[:32, 2, :], kvT_r[:32, 2, :])
        nc.sync.dma_start(kva_sb[:], kva[:].rearrange("(st p) l -> p st l", p=P))

        # ---- phase A: q projection, transposed: qT[l, t] ----------------
        # qT = Wd_w.T.T @ x.T per l-tile, K = C (8 k-steps), N = 512
        for ts in range(2):
            for lt in range(3):
                lp = P if lt < 2 else L - 2 * P   # 128,128,32
                pq = ps_q.tile([P, 512], f32)
                for ct in range(C // P):
                    nc.tensor.matmul(
                        pq[:lp],
                        lhsT=wdT_sb[:, ct, lt * P:lt * P + lp],
                        rhs=xh[ts][:, ct, :],
                        start=(ct == 0),
                        stop=(ct == C // P - 1),
                    )
                # psum f32 -> +bias -> bf16 qT
                nc.scalar.activation(
                    qT_sb[:lp, lt, ts * 512:(ts + 1) * 512],
                    pq[:lp],
                    Ident,
                    bias=bias_sb[:lp, lt:lt + 1],
                    scale=1.0,
                )

        # ---- phase B: attention slots -----------------------------------
        idx = 0
        for j in range(NSLOT):
            n = SLOTS[j]
            t0 = j * CHUNK
            po = [ps_o.tile([P, LA], f32, tag=f"po{tt}", name=f"po_{j}_{tt}")
                  for tt in range(2)]
            for k in range(n):
                ph = ps_h.tile([P, CHUNK], f32)
                for lt in range(3):
                    lp = P if lt < 2 else L - 2 * P
                    nc.tensor.matmul(
                        ph,
                        lhsT=kvT_sb[:lp, lt, k * P:(k + 1) * P],
                        rhs=qT_sb[:lp, lt, t0:t0 + CHUNK],
                        start=(lt == 0),
                        stop=(lt == 2),
                    )
                ye = sb_y.tile([P, CHUNK], bf)
                nc.scalar.activation(ye, ph, Exp, scale=0.125)
                mk = sb_m.tile([P, CHUNK], bf)
                nc.vector.tensor_scalar(
                    mk, dmat_sb, theta_sb[:, idx:idx + 1], None,
                    op0=mybir.AluOpType.is_ge,
                )
                nc.vector.tensor_tensor(ye, ye, mk, op=mybir.AluOpType.mult)
                for tt in range(2):
                    nc.tensor.matmul(
                        po[tt],
                        lhsT=ye[:, tt * P:(tt + 1) * P],
                        rhs=kva_sb[:, k, :],
                        start=(k == 0),
                        stop=(k == n - 1),
                    )
                idx += 1
            for tt in range(2):
                rec = sb_r.tile([P, 1], f32)
                nc.vector.reciprocal(rec, po[tt][:, L:LA])
                ob = sb_o.tile([P, L], f32)
                nc.vector.tensor_scalar_mul(ob, po[tt][:, 0:L], rec)
                trow = t0 + tt * P
                nc.sync.dma_start(out[trow:trow + P, :], ob[:])

    nc.finalize()
    return nc


def _get_program():
    global _cached_nc
    if _cached_nc is None:
        _cached_nc = _build_program()
    return _cached_nc


def _prep_inputs(x, Wdkv, Wd_w, Wd_b):
    """Host-side shard prep: transposes, bf16 casts, masks, thetas."""
    x = np.asarray(x, np.float32)
    Wdkv = np.asarray(Wdkv, np.float32)
    Wd_w = np.asarray(Wd_w, np.float32)
    Wd_b = np.asarray(Wd_b, np.float32)

    wdT = np.ascontiguousarray(Wd_w.T).astype(BF16)          # [C, L]
    bias = np.zeros((P, 3), np.float32)
    bias[:, 0] = Wd_b[0:P]
    bias[:, 1] = Wd_b[P:2 * P]
    bias[:L - 2 * P, 2] = Wd_b[2 * P:L]

    dmat = (np.arange(CHUNK, dtype=np.float32)[None, :]
            - np.arange(P, dtype=np.float32)[:, None]).astype(BF16)

    xT_b = np.ascontiguousarray(x.transpose(0, 2, 1)).astype(BF16)   # [B, C, T]
    kvT_b = np.zeros((B, 3 * P, T), BF16)
    for b in range(B):
        kvT_b[b, :L, :] = Wdkv[b].T.astype(BF16)
    kva_b = np.concatenate(
        [Wdkv, np.ones((B, T, 1), np.float32)], axis=2).astype(BF16)  # [B,T,LA]

    in_maps = []
    for core in range(N_CORES):
        b, h = divmod(core, 2)
        chunks = CHUNKS_H[h]
        cols = np.concatenate(
            [np.arange(c * CHUNK, (c + 1) * CHUNK) for c in chunks])
        theta = np.zeros((P, NTHETA), np.float32)
        idx = 0
        for j, c in enumerate(chunks):
            for k in range(SLOTS[j]):
                theta[:, idx] = 128.0 * k - float(CHUNK) * c
                idx += 1
        in_maps.append({
            "xT": np.ascontiguousarray(xT_b[b][:, cols]),
            "wdT": wdT,
            "bias": bias,
            "kvT": kvT_b[b],
            "kva": kva_b[b],
            "dmat": dmat,
            "theta": theta,
        })
    return in_maps


def _scatter_outputs(results):
    out = np.zeros((B, T, L), np.float32)
    for core in range(N_CORES):
        b, h = divmod(core, 2)
        chunks = CHUNKS_H[h]
        o = np.asarray(results[core]["out"], np.float32)
        for j, c in enumerate(chunks):
            out[b, c * CHUNK:(c + 1) * CHUNK, :] = o[j * CHUNK:(j + 1) * CHUNK]
    return out


def kernel(x, Wdkv, Wd_w, Wd_b, _trace=False):
    nc = _get_program()
    in_maps = _prep_inputs(x, Wdkv, Wd_w, Wd_b)
    res = run_bass_kernel_spmd(nc, in_maps, list(range(N_CORES)), trace=_trace)
    out = _scatter_outputs(res.results)
    if _trace:
        kernel.last_exec_time_ns = res.exec_time_ns
        kernel.last_results = res
    return out


kernel.last_exec_time_ns = None
kernel.last_results = None
